# revision 6
# baseline (speedup 1.0000x reference)
"""ChebyKAN layer (degree-7) on 8 Trainium2 NeuronCores.

out[b,o] = sum_{i,d} T_d(tanh(x[b,i])) * C[o,i,d]  +  x @ BW.T

V2 strategy:
  - Data-parallel over batch: 16384 rows -> 8 cores x 2048.
  - T_0 == 1 contribution folded into a host-precomputed bias[o].
  - Cheby matmuls (7/8 of the FLOPs) run in fp8e4m3 with
    perf_mode=DoubleRow (2 fp8 MACs/cell/cycle, K=256 per matmul);
    coeffs are host-prescaled by 2**16 for fp8 representability.
    The base matmul runs in float32r with base_weight prescaled by
    the same 2**16 so both accumulate into one PSUM tile; the
    eviction rescales by 2**-16 and adds the bias.
  - Chebyshev basis is computed in bf16 on DVE (2x mode), cast to
    fp8 pair-interleaved tiles on ACT, once per batch super-tile
    (reused across both o-half passes).
  - out_features live on PSUM partitions: x ships pre-transposed
    (xT) and outT is transposed back on the host.
"""

import numpy as np

import concourse.mybir as mybir
from concourse import bacc, tile
from concourse.bass_utils import run_bass_kernel_spmd

IN_F = 1024
OUT_F = 1024
DEG = 7
N_CORES = 8
SC = float(2 ** 16)

F32 = mybir.dt.float32
F32R = mybir.dt.float32r
BF16 = mybir.dt.bfloat16
FP8 = mybir.dt.float8e4
ALU = mybir.AluOpType
ACTF = mybir.ActivationFunctionType
DR = mybir.MatmulPerfMode.DoubleRow


def _build_program(b_core: int, n_cores: int = N_CORES):
    bsup = min(1024, b_core)
    assert b_core % bsup == 0
    n_bs = b_core // bsup
    F = bsup
    n_half = (F + 511) // 512
    n_ci = IN_F // 128            # 8
    n_pair = n_ci // 2            # 4
    n_oh = 2

    nc = bacc.Bacc("TRN2", target_bir_lowering=False, debug=False,
                   num_devices=n_cores)
    xT = nc.dram_tensor("xT", [IN_F, b_core], F32R, kind="ExternalInput")
    w8 = nc.dram_tensor("w8", [n_oh, n_pair, 128, DEG * 2 * 512], FP8,
                        kind="ExternalInput")
    wb = nc.dram_tensor("wb", [n_oh, n_ci, 128, 512], F32R,
                        kind="ExternalInput")
    biasm = nc.dram_tensor("biasm", [128, 8], F32, kind="ExternalInput")
    outT = nc.dram_tensor("outT", [OUT_F, b_core], F32, kind="ExternalOutput")

    with tile.TileContext(nc) as tc:
        with (
            tc.tile_pool(name="const", bufs=1) as cpool,
            tc.tile_pool(name="xp", bufs=12) as xpool,
            tc.tile_pool(name="bwork", bufs=12) as kpool,
            tc.tile_pool(name="t8", bufs=7 * n_pair + 7) as t8pool,
            tc.tile_pool(name="w8p", bufs=2) as wpool,
            tc.tile_pool(name="wbp", bufs=4) as wbpool,
            tc.tile_pool(name="op", bufs=4) as opool,
            tc.tile_pool(name="ps", bufs=4, space="PSUM") as ppool,
        ):
            bias_sb = cpool.tile([128, 8], F32)
            nc.sync.dma_start(bias_sb[:], biasm[:, :])

            for bs in range(n_bs):
                # ---- Phase A: bf16 Chebyshev basis -> fp8 interleaved ----
                t8 = {}
                for pair in range(n_pair):
                    for d in range(1, DEG + 1):
                        t8[(pair, d)] = t8pool.tile(
                            [128, 2, F], FP8, tag="t8",
                            name=f"t8_{bs}_{pair}_{d}")
                xts = []
                for ci in range(n_ci):
                    pair, plane = divmod(ci, 2)
                    xt = xpool.tile([128, F], F32R, tag="x",
                                    name=f"x_{bs}_{ci}")
                    nc.sync.dma_start(
                        xt[:], xT[ci * 128:(ci + 1) * 128,
                                  bs * F:(bs + 1) * F])
                    xts.append(xt)
                    xnb = kpool.tile([128, F], BF16, tag="bw")
                    nc.scalar.activation(xnb[:], xt[:].bitcast(F32),
                                         ACTF.Tanh)
                    nc.scalar.copy(t8[(pair, 1)][:, plane, :], xnb[:])
                    m2 = kpool.tile([128, F], BF16, tag="bw")
                    nc.vector.tensor_mul(m2[:], xnb[:], xnb[:])
                    t2 = kpool.tile([128, F], BF16, tag="bw")
                    nc.vector.tensor_scalar(t2[:], m2[:], 2.0, 1.0,
                                            ALU.mult, ALU.subtract)
                    nc.scalar.copy(t8[(pair, 2)][:, plane, :], t2[:])
                    prev2, prev1 = xnb, t2
                    for d in range(3, DEG + 1):
                        md = kpool.tile([128, F], BF16, tag="bw")
                        nc.vector.tensor_mul(md[:], xnb[:], prev1[:])
                        td = kpool.tile([128, F], BF16, tag="bw")
                        nc.vector.scalar_tensor_tensor(
                            td[:], md[:], 2.0, prev2[:],
                            ALU.mult, ALU.subtract)
                        nc.scalar.copy(t8[(pair, d)][:, plane, :], td[:])
                        prev2, prev1 = prev1, td

                # ---- Phase B: matmuls ----
                for oh in range(n_oh):
                    po = [ppool.tile([128, F], F32, tag="ps",
                                     name=f"po_{bs}_{oh}_{i}")
                          for i in range(4)]
                    # base (fp32r) matmuls first: they only need x, so
                    # the PE has work while DVE/ACT produce the basis,
                    # and x tiles retire early in each pass.
                    for ci in range(n_ci):
                        wbt = wbpool.tile([128, 512], F32R, tag="wb")
                        nc.sync.dma_start(wbt[:], wb[oh, ci, :, :])
                        for o4 in range(4):
                            for h in range(n_half):
                                c0 = h * 512
                                c1 = min(c0 + 512, F)
                                nc.tensor.matmul(
                                    po[o4][:, c0:c1],
                                    wbt[:, o4 * 128:(o4 + 1) * 128],
                                    xts[ci][:, c0:c1],
                                    start=(ci == 0),
                                    stop=False)
                    for pair in range(n_pair):
                        wm = wpool.tile([128, DEG * 2 * 512], FP8, tag="w8")
                        nc.sync.dma_start(wm[:], w8[oh, pair, :, :])
                        wmv = wm[:].rearrange("p (d two o) -> p d two o",
                                              d=DEG, two=2)
                        for o4 in range(4):
                            for d in range(1, DEG + 1):
                                lhsT = wmv[:, d - 1, :,
                                           o4 * 128:(o4 + 1) * 128]
                                for h in range(n_half):
                                    c0 = h * 512
                                    c1 = min(c0 + 512, F)
                                    nc.tensor.matmul(
                                        po[o4][:, c0:c1],
                                        lhsT,
                                        t8[(pair, d)][:, :, c0:c1],
                                        start=False,
                                        stop=(pair == n_pair - 1
                                              and d == DEG),
                                        perf_mode=DR)

                    for o4 in range(4):
                        oc = oh * 4 + o4
                        ob = opool.tile([128, F], F32, tag="o")
                        nc.vector.tensor_scalar(
                            ob[:], po[o4][:], 1.0 / SC,
                            bias_sb[:, oc:oc + 1], ALU.mult, ALU.add)
                        nc.sync.dma_start(
                            outT[oc * 128:(oc + 1) * 128,
                                 bs * F:(bs + 1) * F], ob[:])
    nc.compile()
    return nc


def _prep_weights(cheby_coeffs: np.ndarray, base_weight: np.ndarray):
    C = np.asarray(cheby_coeffs, dtype=np.float32)
    BW = np.asarray(base_weight, dtype=np.float32)
    # cheby fp8 mega-tiles: [oh, pair, k, d(1..7), plane, o(512)]
    Cs = (C * SC).reshape(2, 512, 4, 2, 128, DEG + 1)  # [oh,o,pair,plane,k,dg]
    w8 = np.ascontiguousarray(
        Cs[:, :, :, :, :, 1:].transpose(0, 2, 4, 5, 3, 1)
    ).astype(mybir.dt.np(FP8))
    w8 = np.ascontiguousarray(w8.reshape(2, 4, 128, DEG * 2 * 512))
    # base fp32r: [oh, ci, k, o(512)], prescaled
    wbs = (BW.T * SC).reshape(8, 128, 2, 512)          # [ci,k,oh,o]
    wb = np.ascontiguousarray(wbs.transpose(2, 0, 1, 3))
    bias = C[:, :, 0].sum(axis=1)
    biasm = np.ascontiguousarray(bias.reshape(8, 128).T)
    return w8, wb, biasm


_PROGRAM_CACHE = {}


def _make_in_maps(x, cheby_coeffs, base_weight):
    x = np.asarray(x, dtype=np.float32)
    b_core = x.shape[0] // N_CORES
    w8, wb, biasm = _prep_weights(cheby_coeffs, base_weight)
    in_maps = []
    for c in range(N_CORES):
        xs = x[c * b_core:(c + 1) * b_core]
        in_maps.append({
            "xT": np.ascontiguousarray(xs.T),
            "w8": w8,
            "wb": wb,
            "biasm": biasm,
        })
    return in_maps


def kernel(x: np.ndarray, cheby_coeffs: np.ndarray,
           base_weight: np.ndarray) -> np.ndarray:
    x = np.asarray(x, dtype=np.float32)
    b_full = x.shape[0]
    assert b_full % N_CORES == 0
    b_core = b_full // N_CORES

    key = (b_core, N_CORES)
    if key not in _PROGRAM_CACHE:
        _PROGRAM_CACHE[key] = _build_program(b_core)
    nc = _PROGRAM_CACHE[key]

    in_maps = _make_in_maps(x, cheby_coeffs, base_weight)
    res = run_bass_kernel_spmd(nc, in_maps, core_ids=list(range(N_CORES)))
    out = np.empty((b_full, OUT_F), dtype=np.float32)
    for c in range(N_CORES):
        out[c * b_core:(c + 1) * b_core] = res.results[c]["outT"].T
    return out
